# revision 17
# baseline (speedup 1.0000x reference)
"""Trainium2 Bass kernel for nn_Decoder (fc + 3-layer GRU + mask).

Strategy: data-parallel over batch B=32 across 8 cores (4 samples/core),
with the three GRU layers WAVEFRONTED on each core: slot j processes
layer0 chunk j, layer1 chunk j-1, layer2 chunk j-2 (TC=16 steps/chunk),
with per-step emission interleaving so each layer's serial gate chain
(ACT/DVE latency ~2us/step) hides under the other layers' PE matmuls.
Inter-layer chunks are handed off directly through the per-layer hbf
SBUF step buffers (WAR deps make the consumer's gx reads precede the
producer's overwrites) — no DRAM round-trip.

All activations are Sigmoid (single act table, no per-step reload):
h'-space recurrence with h' = (h+1)/2, s' = sigma(2*aN) = (tanh+1)/2,
w = sigma(-az) = 1-z (z weight rows pre-negated host-side):
  h'_new = (1-w) h'_prev + w s'
Weights fold the change of variables: W -> 2W, bias -= rowsum(W).

PSUM pg[l] [128, 16, TC, BS] f32 per layer (4KB/partition x3):
  j 0..7  : gx_rz + biases + gh_rz   (step MMs accumulate; z negated)
  j 8..11 : gx_n + b_ih_n            (read-only per step)
  j 12..15: b_hh_n + gh_n            (step MMs accumulate)
"""

import os
import sys
from contextlib import ExitStack

for _p in ("/opt/trn_rl_repo",):
    if _p not in sys.path:
        sys.path.insert(0, _p)

import numpy as np
import ml_dtypes

import concourse.bass as bass
import concourse.bacc as bacc
import concourse.mybir as mybir
import concourse.tile as tile
from concourse import bass_utils

BF = np.float16
F32 = np.float32
dt = mybir.dt

NCORES = 8
B, T = 32, 512
BS = B // NCORES          # 4 samples per core
TC = 16                   # time-steps per chunk
NCHUNK = T // TC          # 32
HID = 1024                # layer-0 input dim
H = 512                   # GRU hidden
G3 = 3 * H                # 1536
MCH = G3 // 128           # 12 gate-dim chunks: r0..3 z0..3 n0..3
KC = H // 128             # 4 hidden chunks
KC0 = HID // 128          # 8 input chunks for layer 0
ABLATE = ""               # "nogates"
RFULL = 0                 # device-side repeats of the full body

Sigmoid = mybir.ActivationFunctionType.Sigmoid
Relu = mybir.ActivationFunctionType.Relu
Alu = mybir.AluOpType


def _declare_io(nc):
    d = {}
    inp = lambda n, s, t: nc.dram_tensor(n, s, t, kind="ExternalInput").ap()
    d["chordT"] = inp("chordT", [KC0, 128, T, BS], dt.float16)
    d["zT"] = inp("zT", [2, 128, BS], dt.float16)
    d["fcwT"] = inp("fcwT", [2, 128, HID], dt.float16)
    d["fcb"] = inp("fcb", [KC0, 128, 1], dt.float32)
    d["wihT0"] = inp("wihT0", [KC0, 128, G3], dt.float16)
    d["wihT1"] = inp("wihT1", [KC, 128, G3], dt.float16)
    d["wihT2"] = inp("wihT2", [KC, 128, G3], dt.float16)
    d["whhT0"] = inp("whhT0", [KC, 128, G3], dt.float16)
    d["whhT1"] = inp("whhT1", [KC, 128, G3], dt.float16)
    d["whhT2"] = inp("whhT2", [KC, 128, G3], dt.float16)
    d["biasT"] = inp("biasT", [4, 3 * 16 * 128], dt.float16)
    d["onehotT"] = inp("onehotT", [4, TC * BS], dt.float16)
    d["iota"] = inp("iota", [128, T], dt.float32)
    d["seqrep"] = inp("seqrep", [128, BS], dt.float32)
    d["outT"] = nc.dram_tensor("outT", [128, T, KC, BS], dt.float32,
                               kind="ExternalOutput").ap()
    return d


def _build_program(debug=False):
    nc = bacc.Bacc("TRN2", target_bir_lowering=False, debug=debug,
                   num_devices=NCORES)
    io = _declare_io(nc)
    with tile.TileContext(nc) as tc:
        _emit(tc, io)
    nc.compile()
    return nc


def _emit(tc, io):
    nc = tc.nc
    ctx = ExitStack()
    const = ctx.enter_context(tc.tile_pool(name="const", bufs=1))
    stream = ctx.enter_context(tc.tile_pool(name="stream", bufs=3))
    tmp = ctx.enter_context(tc.tile_pool(name="tmp", bufs=3))
    outp = ctx.enter_context(tc.tile_pool(name="outp", bufs=2))
    pgp = ctx.enter_context(tc.tile_pool(name="pg", bufs=1, space="PSUM"))

    # ---- persistent SBUF tensors -------------------------------------
    wih = [const.tile([128, KC0, G3], dt.float16, tag="wih0", name="wih0"),
           const.tile([128, KC, G3], dt.float16, tag="wih1", name="wih1"),
           const.tile([128, KC, G3], dt.float16, tag="wih2", name="wih2")]
    whh = [const.tile([128, KC, G3], dt.float16, tag=f"whh{l}",
                      name=f"whh{l}") for l in range(3)]
    mask = const.tile([128, T, BS], dt.float16, tag="mask")
    biasb = const.tile([4, 3, 16, 128], dt.float16, tag="biasb")
    onehot = const.tile([4, TC * BS], dt.float16, tag="onehot")
    fc_hT = const.tile([128, KC0, BS], dt.float16, tag="fchT")
    gfcT = const.tile([4, MCH, 128], dt.float16, tag="gfcT")
    hbf = [const.tile([128, TC, KC, BS], dt.float16, tag=f"hbf{l}",
                      name=f"hbf{l}") for l in range(3)]

    # ---- load weights / constants ------------------------------------
    for kc in range(KC0):
        nc.sync.dma_start(wih[0][:, kc, :], io["wihT0"][kc])
    for l in (1, 2):
        for kc in range(KC):
            nc.sync.dma_start(wih[l][:, kc, :], io[f"wihT{l}"][kc])
    for l in range(3):
        for kc in range(KC):
            nc.sync.dma_start(whh[l][:, kc, :], io[f"whhT{l}"][kc])
    nc.sync.dma_start(
        biasb[:].rearrange("p l j g -> p (l j g)"), io["biasT"])
    nc.sync.dma_start(onehot[:], io["onehotT"])

    # ---- prologue: mask, fc, gfcT ------------------------------------
    with ExitStack() as pctx:
        psb = pctx.enter_context(tc.tile_pool(name="psb", bufs=2))
        pps = pctx.enter_context(tc.tile_pool(name="pps", bufs=1,
                                              space="PSUM"))

        iota_sb = psb.tile([128, T], dt.float32, tag="iota")
        seq_sb = psb.tile([128, BS], dt.float32, tag="seq")
        nc.sync.dma_start(iota_sb[:], io["iota"])
        nc.sync.dma_start(seq_sb[:], io["seqrep"])
        for b in range(BS):
            # {0,2}-valued: out = 2*(t < seq); folds the h = 2h'-1 output
            # rescale into the mask (h'-space kernel)
            nc.vector.tensor_scalar(mask[:, :, b], iota_sb[:],
                                    seq_sb[:, b:b + 1], 2.0, op0=Alu.is_lt,
                                    op1=Alu.mult)

        z_sb = psb.tile([128, 2, BS], dt.float16, tag="zsb")
        fcw_sb = psb.tile([128, 2, HID], dt.float16, tag="fcw")
        fcb_sb = psb.tile([128, KC0], dt.float32, tag="fcb")
        for kc in range(2):
            nc.sync.dma_start(z_sb[:, kc, :], io["zT"][kc])
            nc.sync.dma_start(fcw_sb[:, kc, :], io["fcwT"][kc])
        for hc in range(KC0):
            nc.sync.dma_start(fcb_sb[:, hc:hc + 1], io["fcb"][hc])
        for hc in range(KC0):
            pfc = pps.tile([128, BS], dt.float32, tag="pfc")
            for kc in range(2):
                nc.tensor.matmul(pfc[:], fcw_sb[:, kc, hc * 128:(hc + 1) * 128],
                                 z_sb[:, kc, :], start=(kc == 0), stop=(kc == 1))
            nc.scalar.activation(fc_hT[:, hc, :], pfc[:], Relu,
                                 bias=fcb_sb[:, hc:hc + 1], scale=1.0)
        # gfcT[b, m*128+g] = (fc_h^T @ w_ih0^T): layer-0 time-constant term
        for m in range(MCH):
            pgf = pps.tile([4, 128], dt.float32, tag="pgf")
            for kc in range(KC0):
                nc.tensor.matmul(pgf[:], fc_hT[:, kc, :],
                                 wih[0][:, kc, m * 128:(m + 1) * 128],
                                 start=(kc == 0), stop=(kc == KC0 - 1))
            nc.vector.tensor_copy(gfcT[:, m, :], pgf[:])

    # ---- wavefront slot loop -----------------------------------------
    def layer_setup(l, ci):
        """Chunk setup for layer l at chunk offset ci (element units):
        gx GEMM + bias (+fc for l=0) seeded into PSUM."""
        pg = pgp.tile([128, 16, TC, BS], dt.float32, tag=f"pg{l}")
        if l == 0:
            chd = stream.tile([128, TC, KC0, BS], dt.float16, tag="chd")
            for kc in range(KC0):
                nc.sync.dma_start(chd[:, :, kc, :],
                                  io["chordT"][kc, :, bass.ds(ci, TC), :])
            srcap = lambda kc: chd[:, :, kc, :]
            kcl = KC0
        else:
            srcap = lambda kc: hbf[l - 1][:, :, kc, :]
            kcl = KC
        # pg is 4KB/partition = 2 PSUM banks (j0..7 / j8..15); start=True
        # clears the WHOLE bank, so only the first writer of each bank
        # (gx m=0 and m=8, kc=0) may set it.
        for m in range(MCH):
            for kc in range(kcl):
                st = (kc == 0) and (m in (0, 8))
                nc.tensor.matmul(
                    pg[:, m, :, :],
                    wih[l][:, kc, m * 128:(m + 1) * 128],
                    srcap(kc), start=st, stop=False,
                    skip_group_check=True)
        for j in range(16):
            nc.tensor.matmul(
                pg[:, j, :, :], biasb[:, l, j, :], onehot[:],
                start=False, stop=False, skip_group_check=True)
        if l == 0:
            for m in range(MCH):
                nc.tensor.matmul(
                    pg[:, m, :, :], gfcT[:, m, :], onehot[:],
                    start=False, stop=False, skip_group_check=True)
        return pg

    def layer_tick(l, s, pg):
        sp = (s - 1) % TC
        if ABLATE == "nochain":
            sp = TC - 1
        rz = tmp.tile([128, 8, BS], dt.float32, tag=f"rz{l}")
        # PE order: r-block, n-block (two m-halves), z-block; all kc-outer
        # so the next tick's kc01 MMs only need the kc01 half of h.
        # sigma_r fires under the n-block; the n-chain (rn,aN,sigma2,delta)
        # runs per m-half under the z-block; the post-z tail is only
        # sigma_w -> w*delta -> h per half.  h' = h'_prev + w*(s'-h'_prev).
        mm = lambda j, m, kc: nc.tensor.matmul(
            pg[:, j, s, :],
            whh[l][:, kc, m * 128:(m + 1) * 128],
            hbf[l][:, sp, kc, :],
            start=False, stop=(s == TC - 1 and kc == KC - 1),
            skip_group_check=True)
        for kc in range(KC):          # r-block -> pg[0:4]
            for m in range(4):
                mm(m, m, kc)
        gates = ABLATE != "nogates"
        if gates:
            nc.scalar.activation(rz[:, 0:4, :], pg[:, 0:4, s, :], Sigmoid)
        rn = tmp.tile([128, KC, BS], dt.float32, tag=f"rn{l}")
        aN = tmp.tile([128, KC, BS], dt.float32, tag=f"aN{l}")
        n = tmp.tile([128, KC, BS], dt.float32, tag=f"n{l}")
        d = tmp.tile([128, KC, BS], dt.float32, tag=f"d{l}")
        wd = tmp.tile([128, KC, BS], dt.float32, tag=f"wd{l}")
        for ha, hb in ((0, 2), (2, 4)):   # n-block halves -> pg[12+..]
            for kc in range(KC):
                for m in (8 + ha, 9 + ha):
                    mm(m + 4, m, kc)
            if gates:
                nc.vector.tensor_mul(rn[:, ha:hb, :],
                                     pg[:, 12 + ha:12 + hb, s, :],
                                     rz[:, ha:hb, :])
                nc.vector.tensor_add(aN[:, ha:hb, :], rn[:, ha:hb, :],
                                     pg[:, 8 + ha:8 + hb, s, :])
                nc.scalar.activation(n[:, ha:hb, :], aN[:, ha:hb, :], Sigmoid,
                                     scale=2.0)
                nc.vector.tensor_sub(d[:, ha:hb, :], n[:, ha:hb, :],
                                     hbf[l][:, sp, ha:hb, :])
        for kc in range(KC):          # z-block -> pg[4:8]
            for m in range(4, 8):
                mm(m, m, kc)
        if not gates:
            return
        nc.scalar.activation(rz[:, 4:8, :], pg[:, 4:8, s, :], Sigmoid)
        for ha, hb in ((0, 2), (2, 4)):
            nc.vector.tensor_mul(wd[:, ha:hb, :], rz[:, 4 + ha:4 + hb, :],
                                 d[:, ha:hb, :])
            nc.vector.tensor_add(hbf[l][:, s, ha:hb, :],
                                 hbf[l][:, sp, ha:hb, :], wd[:, ha:hb, :])

    def out_epilogue(ci):
        mch = stream.tile([128, TC, 1, BS], dt.float16, tag="maskch")
        nc.sync.dma_start(mch[:, :, 0, :], mask[:, bass.ds(ci, TC), :])
        # out = (2h'-1)*m = (h'-0.5)*mask2 with mask2 in {0,2}
        uo = outp.tile([128, TC, KC, BS], dt.float16, tag="uout")
        nc.vector.tensor_scalar_sub(uo[:], hbf[2][:], 0.5)
        osb = outp.tile([128, TC, KC, BS], dt.float32, tag="osb")
        nc.vector.tensor_mul(osb[:], uo[:],
                             mch[:].broadcast_to([128, TC, KC, BS]))
        nc.sync.dma_start(io["outT"][:, bass.ds(ci, TC), :, :], osb[:])

    def slot(c0, c1, c2):
        pgs = {}
        if c0 is not None:
            pgs[0] = layer_setup(0, c0)
        if c1 is not None:
            pgs[1] = layer_setup(1, c1)
        if c2 is not None:
            pgs[2] = layer_setup(2, c2)
        for s in range(TC):
            for l in (0, 1, 2):
                if l in pgs:
                    layer_tick(l, s, pgs[l])
        if c2 is not None:
            out_epilogue(c2)

    full_cm = tc.For_i(0, RFULL, 1, name="rfull") if RFULL else None
    if full_cm is not None:
        full_cm.__enter__()

    for l in range(3):
        # h'-space: h=0 -> h'=0.5
        nc.gpsimd.memset(hbf[l][:, TC - 1, :, :], 0.5)

    slot(0, None, None)
    slot(TC, 0, None)
    hint = (mybir.EngineType.PE,)
    with tc.For_i(2 * TC, T, TC, hint_engines=hint, name="slots") as i:
        slot(i, i - TC, i - 2 * TC)
    slot(None, T - TC, T - 2 * TC)
    slot(None, None, T - TC)

    if full_cm is not None:
        full_cm.__exit__(None, None, None)
    ctx.close()


_CACHE = {}


def _get_program():
    if "nc" not in _CACHE:
        _CACHE["nc"] = _build_program()
    return _CACHE["nc"]


def _prep_shared(fc_w, fc_b, ws):
    sh = {}
    sh["fcwT"] = np.ascontiguousarray(
        fc_w.T.reshape(2, 128, HID)).astype(BF)
    sh["fcb"] = np.ascontiguousarray(fc_b.reshape(KC0, 128, 1)).astype(F32)
    # h'-space (h' = (h+1)/2): hidden-state inputs are h', so W -> 2W and
    # bias -= rowsum(W). Layer-0's input x0 is NOT h'-space (wih0 unscaled).
    # The z-gate rows (512:1024) are then negated so the PSUM holds -az and
    # a single sigmoid over rows 0:8 yields [r, w=1-z] in one ACT op.
    for l in range(3):
        w_ih, w_hh, _, _ = ws[l]
        kcl = KC0 if l == 0 else KC
        wih_eff = (w_ih if l == 0 else 2.0 * w_ih).copy()
        whh_eff = 2.0 * w_hh
        wih_eff[H:2 * H] *= -1.0
        whh_eff[H:2 * H] *= -1.0
        sh[f"wihT{l}"] = np.ascontiguousarray(
            wih_eff.T.reshape(kcl, 128, G3)).astype(BF)
        sh[f"whhT{l}"] = np.ascontiguousarray(
            whh_eff.T.reshape(KC, 128, G3)).astype(BF)
    bt = np.zeros((3, 16, 128), F32)
    for l in range(3):
        w_ih, w_hh, b_ih, b_hh = ws[l]
        bi_eff = b_ih - (0.0 if l == 0 else w_ih.sum(1))
        bh_eff = b_hh - w_hh.sum(1)
        bi = bi_eff.reshape(MCH, 128)
        bh = bh_eff.reshape(MCH, 128)
        bt[l, 0:8] = bi[0:8] + bh[0:8]
        bt[l, 4:8] *= -1.0
        bt[l, 8:12] = bi[8:12]
        bt[l, 12:16] = bh[8:12]
    sh["biasT"] = np.broadcast_to(
        bt.reshape(1, -1), (4, 3 * 16 * 128)).astype(BF).copy()
    oh = np.zeros((4, TC, BS), F32)
    for k in range(BS):
        oh[k, :, k] = 1.0
    sh["onehotT"] = oh.reshape(4, TC * BS).astype(BF)
    sh["iota"] = np.broadcast_to(
        np.arange(T, dtype=F32)[None, :], (128, T)).copy()
    return sh


def kernel(z, seq_lens, chord_embedding, fc_w, fc_b,
           w_ih0, w_hh0, b_ih0, b_hh0,
           w_ih1, w_hh1, b_ih1, b_hh1,
           w_ih2, w_hh2, b_ih2, b_hh2):
    z = np.asarray(z, F32)
    chord = np.asarray(chord_embedding, F32)
    seq = np.asarray(seq_lens)
    ws = [(np.asarray(w_ih0, F32), np.asarray(w_hh0, F32),
           np.asarray(b_ih0, F32), np.asarray(b_hh0, F32)),
          (np.asarray(w_ih1, F32), np.asarray(w_hh1, F32),
           np.asarray(b_ih1, F32), np.asarray(b_hh1, F32)),
          (np.asarray(w_ih2, F32), np.asarray(w_hh2, F32),
           np.asarray(b_ih2, F32), np.asarray(b_hh2, F32))]

    in_maps = _make_in_maps(z, seq, chord, np.asarray(fc_w, F32),
                            np.asarray(fc_b, F32), ws)
    res = _execute(in_maps)
    return _assemble(res.results)


def _make_in_maps(z, seq, chord, fc_w, fc_b, ws):
    sh = _prep_shared(fc_w, fc_b, ws)
    in_maps = []
    for c in range(NCORES):
        bs = slice(c * BS, (c + 1) * BS)
        m = dict(sh)
        m["chordT"] = np.ascontiguousarray(
            (chord[bs].transpose(2, 1, 0) / 100.0)
            .reshape(KC0, 128, T, BS)).astype(BF)
        m["zT"] = np.ascontiguousarray(
            z[bs].T.reshape(2, 128, BS)).astype(BF)
        m["seqrep"] = np.broadcast_to(
            seq[bs].astype(F32)[None, :], (128, BS)).copy()
        in_maps.append(m)
    return in_maps


def _execute(in_maps, **kw):
    nc = _get_program()
    return bass_utils.run_bass_kernel_spmd(nc, in_maps, list(range(NCORES)), **kw)


def _assemble(results):
    out = np.empty((B, T, H), F32)
    for c in range(NCORES):
        outT = np.asarray(results[c]["outT"])       # [128,T,KC,BS]
        out[c * BS:(c + 1) * BS] = (
            outT.transpose(3, 1, 2, 0).reshape(BS, T, H))
    return out


# revision 18
# speedup vs baseline: 1.1180x; 1.1180x over previous
"""Trainium2 Bass kernel for nn_Decoder (fc + 3-layer GRU + mask).

Strategy: data-parallel over batch B=32 across 8 cores (4 samples/core),
with the three GRU layers WAVEFRONTED on each core: slot j processes
layer0 chunk j, layer1 chunk j-1, layer2 chunk j-2 (TC=16 steps/chunk),
with per-step emission interleaving so each layer's serial gate chain
(ACT/DVE latency ~2us/step) hides under the other layers' PE matmuls.
Inter-layer chunks are handed off directly through the per-layer hbf
SBUF step buffers (WAR deps make the consumer's gx reads precede the
producer's overwrites) — no DRAM round-trip.

All activations are Sigmoid (single act table, no per-step reload):
h'-space recurrence with h' = (h+1)/2, s' = sigma(2*aN) = (tanh+1)/2,
w = sigma(-az) = 1-z (z weight rows pre-negated host-side):
  h'_new = (1-w) h'_prev + w s'
Weights fold the change of variables: W -> 2W, bias -= rowsum(W).

PSUM pg[l] [128, 16, TC, BS] f32 per layer (4KB/partition x3):
  j 0..7  : gx_rz + biases + gh_rz   (step MMs accumulate; z negated)
  j 8..11 : gx_n + b_ih_n            (read-only per step)
  j 12..15: b_hh_n + gh_n            (step MMs accumulate)
"""

import os
import sys
from contextlib import ExitStack

for _p in ("/opt/trn_rl_repo",):
    if _p not in sys.path:
        sys.path.insert(0, _p)

import numpy as np
import ml_dtypes

import concourse.bass as bass
import concourse.bacc as bacc
import concourse.mybir as mybir
import concourse.tile as tile
from concourse import bass_utils

BF = np.float16
F32 = np.float32
dt = mybir.dt

NCORES = 8
B, T = 32, 512
BS = B // NCORES          # 4 samples per core
TC = 16                   # time-steps per chunk
NCHUNK = T // TC          # 32
HID = 1024                # layer-0 input dim
H = 512                   # GRU hidden
G3 = 3 * H                # 1536
MCH = G3 // 128           # 12 gate-dim chunks: r0..3 z0..3 n0..3
KC = H // 128             # 4 hidden chunks
KC0 = HID // 128          # 8 input chunks for layer 0
ABLATE = ""               # "nogates"
RFULL = 0                 # device-side repeats of the full body

Sigmoid = mybir.ActivationFunctionType.Sigmoid
Relu = mybir.ActivationFunctionType.Relu
Alu = mybir.AluOpType


def _declare_io(nc):
    d = {}
    inp = lambda n, s, t: nc.dram_tensor(n, s, t, kind="ExternalInput").ap()
    d["chordT"] = inp("chordT", [KC0, 128, T, BS], dt.float16)
    d["zT"] = inp("zT", [2, 128, BS], dt.float16)
    d["fcwT"] = inp("fcwT", [2, 128, HID], dt.float16)
    d["fcb"] = inp("fcb", [KC0, 128, 1], dt.float32)
    d["wihT0"] = inp("wihT0", [KC0, 128, G3], dt.float16)
    d["wihT1"] = inp("wihT1", [KC, 128, G3], dt.float16)
    d["wihT2"] = inp("wihT2", [KC, 128, G3], dt.float16)
    d["whhT0"] = inp("whhT0", [KC, 128, G3], dt.float16)
    d["whhT1"] = inp("whhT1", [KC, 128, G3], dt.float16)
    d["whhT2"] = inp("whhT2", [KC, 128, G3], dt.float16)
    d["biasT"] = inp("biasT", [4, 3 * 16 * 128], dt.float16)
    d["onehotT"] = inp("onehotT", [4, TC * BS], dt.float16)
    d["iota"] = inp("iota", [128, T], dt.float32)
    d["seqrep"] = inp("seqrep", [128, BS], dt.float32)
    d["outT"] = nc.dram_tensor("outT", [128, T, KC, BS], dt.float32,
                               kind="ExternalOutput").ap()
    return d


def _build_program(debug=False):
    nc = bacc.Bacc("TRN2", target_bir_lowering=False, debug=debug,
                   num_devices=NCORES)
    io = _declare_io(nc)
    with tile.TileContext(nc) as tc:
        _emit(tc, io)
    nc.compile()
    return nc


def _emit(tc, io):
    nc = tc.nc
    ctx = ExitStack()
    const = ctx.enter_context(tc.tile_pool(name="const", bufs=1))
    stream = ctx.enter_context(tc.tile_pool(name="stream", bufs=3))
    tmp = ctx.enter_context(tc.tile_pool(name="tmp", bufs=3))
    outp = ctx.enter_context(tc.tile_pool(name="outp", bufs=2))
    pgp = ctx.enter_context(tc.tile_pool(name="pg", bufs=1, space="PSUM"))

    # ---- persistent SBUF tensors -------------------------------------
    wih = [const.tile([128, KC0, G3], dt.float16, tag="wih0", name="wih0"),
           const.tile([128, KC, G3], dt.float16, tag="wih1", name="wih1"),
           const.tile([128, KC, G3], dt.float16, tag="wih2", name="wih2")]
    whh = [const.tile([128, KC, G3], dt.float16, tag=f"whh{l}",
                      name=f"whh{l}") for l in range(3)]
    mask = const.tile([128, T, BS], dt.float16, tag="mask")
    biasb = const.tile([4, 3, 16, 128], dt.float16, tag="biasb")
    onehot = const.tile([4, TC * BS], dt.float16, tag="onehot")
    fc_hT = const.tile([128, KC0, BS], dt.float16, tag="fchT")
    gfcT = const.tile([4, MCH, 128], dt.float16, tag="gfcT")
    hbf = [const.tile([128, TC, KC, BS], dt.float16, tag=f"hbf{l}",
                      name=f"hbf{l}") for l in range(3)]

    # ---- load weights / constants ------------------------------------
    for kc in range(KC0):
        nc.sync.dma_start(wih[0][:, kc, :], io["wihT0"][kc])
    for l in (1, 2):
        for kc in range(KC):
            nc.sync.dma_start(wih[l][:, kc, :], io[f"wihT{l}"][kc])
    for l in range(3):
        for kc in range(KC):
            nc.sync.dma_start(whh[l][:, kc, :], io[f"whhT{l}"][kc])
    nc.sync.dma_start(
        biasb[:].rearrange("p l j g -> p (l j g)"), io["biasT"])
    nc.sync.dma_start(onehot[:], io["onehotT"])

    # ---- prologue: mask, fc, gfcT ------------------------------------
    with ExitStack() as pctx:
        psb = pctx.enter_context(tc.tile_pool(name="psb", bufs=2))
        pps = pctx.enter_context(tc.tile_pool(name="pps", bufs=1,
                                              space="PSUM"))

        iota_sb = psb.tile([128, T], dt.float32, tag="iota")
        seq_sb = psb.tile([128, BS], dt.float32, tag="seq")
        nc.sync.dma_start(iota_sb[:], io["iota"])
        nc.sync.dma_start(seq_sb[:], io["seqrep"])
        for b in range(BS):
            # {0,2}-valued: out = 2*(t < seq); folds the h = 2h'-1 output
            # rescale into the mask (h'-space kernel)
            nc.vector.tensor_scalar(mask[:, :, b], iota_sb[:],
                                    seq_sb[:, b:b + 1], 2.0, op0=Alu.is_lt,
                                    op1=Alu.mult)

        z_sb = psb.tile([128, 2, BS], dt.float16, tag="zsb")
        fcw_sb = psb.tile([128, 2, HID], dt.float16, tag="fcw")
        fcb_sb = psb.tile([128, KC0], dt.float32, tag="fcb")
        for kc in range(2):
            nc.sync.dma_start(z_sb[:, kc, :], io["zT"][kc])
            nc.sync.dma_start(fcw_sb[:, kc, :], io["fcwT"][kc])
        for hc in range(KC0):
            nc.sync.dma_start(fcb_sb[:, hc:hc + 1], io["fcb"][hc])
        for hc in range(KC0):
            pfc = pps.tile([128, BS], dt.float32, tag="pfc")
            for kc in range(2):
                nc.tensor.matmul(pfc[:], fcw_sb[:, kc, hc * 128:(hc + 1) * 128],
                                 z_sb[:, kc, :], start=(kc == 0), stop=(kc == 1))
            nc.scalar.activation(fc_hT[:, hc, :], pfc[:], Relu,
                                 bias=fcb_sb[:, hc:hc + 1], scale=1.0)
        # gfcT[b, m*128+g] = (fc_h^T @ w_ih0^T): layer-0 time-constant term
        for m in range(MCH):
            pgf = pps.tile([4, 128], dt.float32, tag="pgf")
            for kc in range(KC0):
                nc.tensor.matmul(pgf[:], fc_hT[:, kc, :],
                                 wih[0][:, kc, m * 128:(m + 1) * 128],
                                 start=(kc == 0), stop=(kc == KC0 - 1))
            nc.vector.tensor_copy(gfcT[:, m, :], pgf[:])

    # ---- wavefront slot loop -----------------------------------------
    def layer_setup(l, ci):
        """Chunk setup for layer l at chunk offset ci (element units):
        gx GEMM + bias (+fc for l=0) seeded into PSUM."""
        pg = pgp.tile([128, 16, TC, BS], dt.float32, tag=f"pg{l}")
        if l == 0:
            chd = stream.tile([128, TC, KC0, BS], dt.float16, tag="chd")
            for kc in range(KC0):
                nc.sync.dma_start(chd[:, :, kc, :],
                                  io["chordT"][kc, :, bass.ds(ci, TC), :])
            srcap = lambda kc: chd[:, :, kc, :]
            kcl = KC0
        else:
            srcap = lambda kc: hbf[l - 1][:, :, kc, :]
            kcl = KC
        # pg is 4KB/partition = 2 PSUM banks (j0..7 / j8..15); start=True
        # clears the WHOLE bank, so only the first writer of each bank
        # (gx m=0 and m=8, kc=0) may set it.
        for m in range(MCH):
            for kc in range(kcl):
                st = (kc == 0) and (m in (0, 8))
                nc.tensor.matmul(
                    pg[:, m, :, :],
                    wih[l][:, kc, m * 128:(m + 1) * 128],
                    srcap(kc), start=st, stop=False,
                    skip_group_check=True)
        for j in range(16):
            nc.tensor.matmul(
                pg[:, j, :, :], biasb[:, l, j, :], onehot[:],
                start=False, stop=False, skip_group_check=True)
        if l == 0:
            for m in range(MCH):
                nc.tensor.matmul(
                    pg[:, m, :, :], gfcT[:, m, :], onehot[:],
                    start=False, stop=False, skip_group_check=True)
        return pg

    def layer_tick(l, s, pg):
        sp = (s - 1) % TC
        if ABLATE == "nochain":
            sp = TC - 1
        rz = tmp.tile([128, 8, BS], dt.float32, tag=f"rz{l}")
        # PE order: rz-block, n-block, both kc-outer so the next tick's
        # kc01 MMs only need the kc01 half of h. sigma_rw fires after the
        # rz block (one ACT op: z rows pre-negated so rz[4:8]=w=1-z) and
        # hides under the n block; whp/u hide under sigma2. The post-MM
        # chain is split in kc halves so h[kc01] lands one half-chain
        # early and the next tick's PE starts sooner.
        for kc in range(KC):
            for m in range(8):
                nc.tensor.matmul(
                    pg[:, m, s, :],
                    whh[l][:, kc, m * 128:(m + 1) * 128],
                    hbf[l][:, sp, kc, :],
                    start=False, stop=(s == TC - 1 and kc == KC - 1),
                    skip_group_check=True)
        if ABLATE != "nogates":
            nc.scalar.activation(rz[:], pg[:, 0:8, s, :], Sigmoid)
        for kc in range(KC):
            for m in range(8, MCH):
                nc.tensor.matmul(
                    pg[:, m + 4, s, :],
                    whh[l][:, kc, m * 128:(m + 1) * 128],
                    hbf[l][:, sp, kc, :],
                    start=False, stop=(s == TC - 1 and kc == KC - 1),
                    skip_group_check=True)
        if ABLATE == "nogates":
            return
        # h'_new = (1-w)h'_prev + w*s',  s' = sigma(2*aN)
        whp = tmp.tile([128, KC, BS], dt.float32, tag=f"whp{l}")
        nc.vector.tensor_mul(whp[:], rz[:, 4:8, :], hbf[l][:, sp, :, :])
        u = tmp.tile([128, KC, BS], dt.float32, tag=f"u{l}")
        nc.vector.tensor_sub(u[:], hbf[l][:, sp, :, :], whp[:])
        rn = tmp.tile([128, KC, BS], dt.float32, tag=f"rn{l}")
        aN = tmp.tile([128, KC, BS], dt.float32, tag=f"aN{l}")
        n = tmp.tile([128, KC, BS], dt.float32, tag=f"n{l}")
        v = tmp.tile([128, KC, BS], dt.float32, tag=f"v{l}")
        for ha, hb in ((0, 2), (2, 4)):
            nc.vector.tensor_mul(rn[:, ha:hb, :], pg[:, 12 + ha:12 + hb, s, :],
                                 rz[:, ha:hb, :])
            nc.vector.tensor_add(aN[:, ha:hb, :], rn[:, ha:hb, :],
                                 pg[:, 8 + ha:8 + hb, s, :])
            nc.scalar.activation(n[:, ha:hb, :], aN[:, ha:hb, :], Sigmoid,
                                 scale=2.0)
            nc.vector.tensor_mul(v[:, ha:hb, :], rz[:, 4 + ha:4 + hb, :],
                                 n[:, ha:hb, :])
            nc.vector.tensor_add(hbf[l][:, s, ha:hb, :], u[:, ha:hb, :],
                                 v[:, ha:hb, :])

    def out_epilogue(ci):
        mch = stream.tile([128, TC, 1, BS], dt.float16, tag="maskch")
        nc.sync.dma_start(mch[:, :, 0, :], mask[:, bass.ds(ci, TC), :])
        # out = (2h'-1)*m = (h'-0.5)*mask2 with mask2 in {0,2}
        uo = outp.tile([128, TC, KC, BS], dt.float16, tag="uout")
        nc.vector.tensor_scalar_sub(uo[:], hbf[2][:], 0.5)
        osb = outp.tile([128, TC, KC, BS], dt.float32, tag="osb")
        nc.vector.tensor_mul(osb[:], uo[:],
                             mch[:].broadcast_to([128, TC, KC, BS]))
        nc.sync.dma_start(io["outT"][:, bass.ds(ci, TC), :, :], osb[:])

    def slot(c0, c1, c2):
        pgs = {}
        if c0 is not None:
            pgs[0] = layer_setup(0, c0)
        if c1 is not None:
            pgs[1] = layer_setup(1, c1)
        if c2 is not None:
            pgs[2] = layer_setup(2, c2)
        for s in range(TC):
            for l in (0, 1, 2):
                if l in pgs:
                    layer_tick(l, s, pgs[l])
        if c2 is not None:
            out_epilogue(c2)

    full_cm = tc.For_i(0, RFULL, 1, name="rfull") if RFULL else None
    if full_cm is not None:
        full_cm.__enter__()

    for l in range(3):
        # h'-space: h=0 -> h'=0.5
        nc.gpsimd.memset(hbf[l][:, TC - 1, :, :], 0.5)

    slot(0, None, None)
    slot(TC, 0, None)
    hint = (mybir.EngineType.PE,)
    with tc.For_i(2 * TC, T, TC, hint_engines=hint, name="slots") as i:
        slot(i, i - TC, i - 2 * TC)
    slot(None, T - TC, T - 2 * TC)
    slot(None, None, T - TC)

    if full_cm is not None:
        full_cm.__exit__(None, None, None)
    ctx.close()


_CACHE = {}


def _get_program():
    if "nc" not in _CACHE:
        _CACHE["nc"] = _build_program()
    return _CACHE["nc"]


def _prep_shared(fc_w, fc_b, ws):
    sh = {}
    sh["fcwT"] = np.ascontiguousarray(
        fc_w.T.reshape(2, 128, HID)).astype(BF)
    sh["fcb"] = np.ascontiguousarray(fc_b.reshape(KC0, 128, 1)).astype(F32)
    # h'-space (h' = (h+1)/2): hidden-state inputs are h', so W -> 2W and
    # bias -= rowsum(W). Layer-0's input x0 is NOT h'-space (wih0 unscaled).
    # The z-gate rows (512:1024) are then negated so the PSUM holds -az and
    # a single sigmoid over rows 0:8 yields [r, w=1-z] in one ACT op.
    for l in range(3):
        w_ih, w_hh, _, _ = ws[l]
        kcl = KC0 if l == 0 else KC
        wih_eff = (w_ih if l == 0 else 2.0 * w_ih).copy()
        whh_eff = 2.0 * w_hh
        wih_eff[H:2 * H] *= -1.0
        whh_eff[H:2 * H] *= -1.0
        sh[f"wihT{l}"] = np.ascontiguousarray(
            wih_eff.T.reshape(kcl, 128, G3)).astype(BF)
        sh[f"whhT{l}"] = np.ascontiguousarray(
            whh_eff.T.reshape(KC, 128, G3)).astype(BF)
    bt = np.zeros((3, 16, 128), F32)
    for l in range(3):
        w_ih, w_hh, b_ih, b_hh = ws[l]
        bi_eff = b_ih - (0.0 if l == 0 else w_ih.sum(1))
        bh_eff = b_hh - w_hh.sum(1)
        bi = bi_eff.reshape(MCH, 128)
        bh = bh_eff.reshape(MCH, 128)
        bt[l, 0:8] = bi[0:8] + bh[0:8]
        bt[l, 4:8] *= -1.0
        bt[l, 8:12] = bi[8:12]
        bt[l, 12:16] = bh[8:12]
    sh["biasT"] = np.broadcast_to(
        bt.reshape(1, -1), (4, 3 * 16 * 128)).astype(BF).copy()
    oh = np.zeros((4, TC, BS), F32)
    for k in range(BS):
        oh[k, :, k] = 1.0
    sh["onehotT"] = oh.reshape(4, TC * BS).astype(BF)
    sh["iota"] = np.broadcast_to(
        np.arange(T, dtype=F32)[None, :], (128, T)).copy()
    return sh


def kernel(z, seq_lens, chord_embedding, fc_w, fc_b,
           w_ih0, w_hh0, b_ih0, b_hh0,
           w_ih1, w_hh1, b_ih1, b_hh1,
           w_ih2, w_hh2, b_ih2, b_hh2):
    z = np.asarray(z, F32)
    chord = np.asarray(chord_embedding, F32)
    seq = np.asarray(seq_lens)
    ws = [(np.asarray(w_ih0, F32), np.asarray(w_hh0, F32),
           np.asarray(b_ih0, F32), np.asarray(b_hh0, F32)),
          (np.asarray(w_ih1, F32), np.asarray(w_hh1, F32),
           np.asarray(b_ih1, F32), np.asarray(b_hh1, F32)),
          (np.asarray(w_ih2, F32), np.asarray(w_hh2, F32),
           np.asarray(b_ih2, F32), np.asarray(b_hh2, F32))]

    in_maps = _make_in_maps(z, seq, chord, np.asarray(fc_w, F32),
                            np.asarray(fc_b, F32), ws)
    res = _execute(in_maps)
    return _assemble(res.results)


def _make_in_maps(z, seq, chord, fc_w, fc_b, ws):
    sh = _prep_shared(fc_w, fc_b, ws)
    in_maps = []
    for c in range(NCORES):
        bs = slice(c * BS, (c + 1) * BS)
        m = dict(sh)
        m["chordT"] = np.ascontiguousarray(
            (chord[bs].transpose(2, 1, 0) / 100.0)
            .reshape(KC0, 128, T, BS)).astype(BF)
        m["zT"] = np.ascontiguousarray(
            z[bs].T.reshape(2, 128, BS)).astype(BF)
        m["seqrep"] = np.broadcast_to(
            seq[bs].astype(F32)[None, :], (128, BS)).copy()
        in_maps.append(m)
    return in_maps


def _execute(in_maps, **kw):
    nc = _get_program()
    return bass_utils.run_bass_kernel_spmd(nc, in_maps, list(range(NCORES)), **kw)


def _assemble(results):
    out = np.empty((B, T, H), F32)
    for c in range(NCORES):
        outT = np.asarray(results[c]["outT"])       # [128,T,KC,BS]
        out[c * BS:(c + 1) * BS] = (
            outT.transpose(3, 1, 2, 0).reshape(BS, T, H))
    return out


# revision 22
# speedup vs baseline: 1.1399x; 1.0196x over previous
"""Trainium2 Bass kernel for nn_Decoder (fc + 3-layer GRU + mask).

Strategy: data-parallel over batch B=32 across 8 cores (4 samples/core),
with the three GRU layers WAVEFRONTED on each core: slot j processes
layer0 chunk j, layer1 chunk j-1, layer2 chunk j-2 (TC=16 steps/chunk),
with per-step emission interleaving so each layer's serial gate chain
(ACT/DVE latency ~2us/step) hides under the other layers' PE matmuls.
Inter-layer chunks are handed off directly through the per-layer hbf
SBUF step buffers (WAR deps make the consumer's gx reads precede the
producer's overwrites) — no DRAM round-trip.

All activations are Sigmoid (single act table, no per-step reload):
h'-space recurrence with h' = (h+1)/2, s' = sigma(2*aN) = (tanh+1)/2,
w = sigma(-az) = 1-z (z weight rows pre-negated host-side):
  h'_new = (1-w) h'_prev + w s'
Weights fold the change of variables: W -> 2W, bias -= rowsum(W).

PSUM pg[l] [128, 16, TC, BS] f32 per layer (4KB/partition x3):
  j 0..7  : gx_rz + biases + gh_rz   (step MMs accumulate; z negated)
  j 8..11 : gx_n + b_ih_n            (read-only per step)
  j 12..15: b_hh_n + gh_n            (step MMs accumulate)
"""

import os
import sys
from contextlib import ExitStack

for _p in ("/opt/trn_rl_repo",):
    if _p not in sys.path:
        sys.path.insert(0, _p)

import numpy as np
import ml_dtypes

import concourse.bass as bass
import concourse.bacc as bacc
import concourse.mybir as mybir
import concourse.tile as tile
from concourse import bass_utils

BF = np.float16
F32 = np.float32
dt = mybir.dt

NCORES = 8
B, T = 32, 512
BS = B // NCORES          # 4 samples per core
TC = 16                   # time-steps per chunk
NCHUNK = T // TC          # 32
HID = 1024                # layer-0 input dim
H = 512                   # GRU hidden
G3 = 3 * H                # 1536
MCH = G3 // 128           # 12 gate-dim chunks: r0..3 z0..3 n0..3
KC = H // 128             # 4 hidden chunks
KC0 = HID // 128          # 8 input chunks for layer 0
ABLATE = ""               # "nogates"
RFULL = 0                 # device-side repeats of the full body

Sigmoid = mybir.ActivationFunctionType.Sigmoid
Relu = mybir.ActivationFunctionType.Relu
Alu = mybir.AluOpType


def _declare_io(nc):
    d = {}
    inp = lambda n, s, t: nc.dram_tensor(n, s, t, kind="ExternalInput").ap()
    d["chordT"] = inp("chordT", [KC0, 128, T, BS], dt.float16)
    d["zT"] = inp("zT", [2, 128, BS], dt.float16)
    d["fcwT"] = inp("fcwT", [2, 128, HID], dt.float16)
    d["fcb"] = inp("fcb", [KC0, 128, 1], dt.float32)
    d["wihT0"] = inp("wihT0", [KC0, 128, G3], dt.float16)
    d["wihT1"] = inp("wihT1", [KC, 128, G3], dt.float16)
    d["wihT2"] = inp("wihT2", [KC, 128, G3], dt.float16)
    d["whhT0"] = inp("whhT0", [KC, 128, G3], dt.float16)
    d["whhT1"] = inp("whhT1", [KC, 128, G3], dt.float16)
    d["whhT2"] = inp("whhT2", [KC, 128, G3], dt.float16)
    d["biasT"] = inp("biasT", [4, 3 * 16 * 128], dt.float16)
    d["onehotT"] = inp("onehotT", [4, TC * BS], dt.float16)
    d["iota"] = inp("iota", [128, T], dt.float32)
    d["seqrep"] = inp("seqrep", [128, BS], dt.float32)
    d["outT"] = nc.dram_tensor("outT", [128, T, KC, BS], dt.float32,
                               kind="ExternalOutput").ap()
    return d


def _build_program(debug=False):
    nc = bacc.Bacc("TRN2", target_bir_lowering=False, debug=debug,
                   num_devices=NCORES)
    io = _declare_io(nc)
    with tile.TileContext(nc) as tc:
        _emit(tc, io)
    nc.compile()
    return nc


def _emit(tc, io):
    nc = tc.nc
    ctx = ExitStack()
    const = ctx.enter_context(tc.tile_pool(name="const", bufs=1))
    stream = ctx.enter_context(tc.tile_pool(name="stream", bufs=3))
    tmp = ctx.enter_context(tc.tile_pool(name="tmp", bufs=3))
    outp = ctx.enter_context(tc.tile_pool(name="outp", bufs=2))
    pgp = ctx.enter_context(tc.tile_pool(name="pg", bufs=1, space="PSUM"))

    # ---- persistent SBUF tensors -------------------------------------
    wih = [const.tile([128, KC0, G3], dt.float16, tag="wih0", name="wih0"),
           const.tile([128, KC, G3], dt.float16, tag="wih1", name="wih1"),
           const.tile([128, KC, G3], dt.float16, tag="wih2", name="wih2")]
    whh = [const.tile([128, KC, G3], dt.float16, tag=f"whh{l}",
                      name=f"whh{l}") for l in range(3)]
    mask = const.tile([128, T, BS], dt.float16, tag="mask")
    biasb = const.tile([4, 3, 16, 128], dt.float16, tag="biasb")
    onehot = const.tile([4, TC * BS], dt.float16, tag="onehot")
    fc_hT = const.tile([128, KC0, BS], dt.float16, tag="fchT")
    gfcT = const.tile([4, MCH, 128], dt.float16, tag="gfcT")
    biasc0 = const.tile([4, 16, 128], dt.float16, tag="biasc0")
    hbf = [const.tile([128, TC, KC, BS], dt.float16, tag=f"hbf{l}",
                      name=f"hbf{l}") for l in range(3)]

    # ---- load weights / constants ------------------------------------
    for kc in range(KC0):
        nc.sync.dma_start(wih[0][:, kc, :], io["wihT0"][kc])
    for l in (1, 2):
        for kc in range(KC):
            nc.sync.dma_start(wih[l][:, kc, :], io[f"wihT{l}"][kc])
    for l in range(3):
        for kc in range(KC):
            nc.sync.dma_start(whh[l][:, kc, :], io[f"whhT{l}"][kc])
    nc.sync.dma_start(
        biasb[:].rearrange("p l j g -> p (l j g)"), io["biasT"])
    nc.sync.dma_start(onehot[:], io["onehotT"])

    # ---- prologue: mask, fc, gfcT ------------------------------------
    with ExitStack() as pctx:
        psb = pctx.enter_context(tc.tile_pool(name="psb", bufs=2))
        pps = pctx.enter_context(tc.tile_pool(name="pps", bufs=1,
                                              space="PSUM"))

        iota_sb = psb.tile([128, T], dt.float32, tag="iota")
        seq_sb = psb.tile([128, BS], dt.float32, tag="seq")
        nc.sync.dma_start(iota_sb[:], io["iota"])
        nc.sync.dma_start(seq_sb[:], io["seqrep"])
        for b in range(BS):
            # {0,2}-valued: out = 2*(t < seq); folds the h = 2h'-1 output
            # rescale into the mask (h'-space kernel)
            nc.vector.tensor_scalar(mask[:, :, b], iota_sb[:],
                                    seq_sb[:, b:b + 1], 2.0, op0=Alu.is_lt,
                                    op1=Alu.mult)

        z_sb = psb.tile([128, 2, BS], dt.float16, tag="zsb")
        fcw_sb = psb.tile([128, 2, HID], dt.float16, tag="fcw")
        fcb_sb = psb.tile([128, KC0], dt.float32, tag="fcb")
        for kc in range(2):
            nc.sync.dma_start(z_sb[:, kc, :], io["zT"][kc])
            nc.sync.dma_start(fcw_sb[:, kc, :], io["fcwT"][kc])
        for hc in range(KC0):
            nc.sync.dma_start(fcb_sb[:, hc:hc + 1], io["fcb"][hc])
        for hc in range(KC0):
            pfc = pps.tile([128, BS], dt.float32, tag="pfc")
            for kc in range(2):
                nc.tensor.matmul(pfc[:], fcw_sb[:, kc, hc * 128:(hc + 1) * 128],
                                 z_sb[:, kc, :], start=(kc == 0), stop=(kc == 1))
            nc.scalar.activation(fc_hT[:, hc, :], pfc[:], Relu,
                                 bias=fcb_sb[:, hc:hc + 1], scale=1.0)
        # gfcT[b, m*128+g] = (fc_h^T @ w_ih0^T): layer-0 time-constant term
        for m in range(MCH):
            pgf = pps.tile([4, 128], dt.float32, tag="pgf")
            for kc in range(KC0):
                nc.tensor.matmul(pgf[:], fc_hT[:, kc, :],
                                 wih[0][:, kc, m * 128:(m + 1) * 128],
                                 start=(kc == 0), stop=(kc == KC0 - 1))
            nc.vector.tensor_copy(gfcT[:, m, :], pgf[:])
        # layer-0 combined per-sample seed: bias + fc term (j 0..11), so
        # the per-slot gfc MMs disappear
        nc.vector.tensor_add(biasc0[:, 0:MCH, :], biasb[:, 0, 0:MCH, :],
                             gfcT[:])
        nc.vector.tensor_copy(biasc0[:, MCH:16, :], biasb[:, 0, MCH:16, :])

    # ---- wavefront slot loop -----------------------------------------
    def layer_setup(l, ci):
        """Chunk setup for layer l at chunk offset ci (element units):
        gx GEMM + bias (+fc for l=0) seeded into PSUM."""
        pg = pgp.tile([128, 16, TC, BS], dt.float32, tag=f"pg{l}")
        if l == 0:
            chd = stream.tile([128, TC, KC0, BS], dt.float16, tag="chd")
            for kc in range(KC0):
                nc.sync.dma_start(chd[:, :, kc, :],
                                  io["chordT"][kc, :, bass.ds(ci, TC), :])
            srcap = lambda kc: chd[:, :, kc, :]
            kcl = KC0
        else:
            srcap = lambda kc: hbf[l - 1][:, :, kc, :]
            kcl = KC
        # pg is 4KB/partition = 2 PSUM banks (j0..7 / j8..15); start=True
        # clears the WHOLE bank, so only the first writer of each bank
        # (gx m=0 and m=8, kc=0) may set it.
        for m in range(MCH):
            for kc in range(kcl):
                st = (kc == 0) and (m in (0, 8))
                nc.tensor.matmul(
                    pg[:, m, :, :],
                    wih[l][:, kc, m * 128:(m + 1) * 128],
                    srcap(kc), start=st, stop=False,
                    skip_group_check=True)
        for j in range(16):
            bsrc = biasc0[:, j, :] if l == 0 else biasb[:, l, j, :]
            nc.tensor.matmul(
                pg[:, j, :, :], bsrc, onehot[:],
                start=False, stop=False, skip_group_check=True)
        return pg

    def layer_tick(l, s, pg):
        sp = (s - 1) % TC
        if ABLATE == "nochain":
            sp = TC - 1
        rz = tmp.tile([128, 8, BS], dt.float32, tag=f"rz{l}")
        # PE order: rz-block, n-block, both kc-outer so the next tick's
        # kc01 MMs only need the kc01 half of h. sigma_rw fires after the
        # rz block (one ACT op: z rows pre-negated so rz[4:8]=w=1-z) and
        # hides under the n block; whp/u hide under sigma2. The post-MM
        # chain is split in kc halves so h[kc01] lands one half-chain
        # early and the next tick's PE starts sooner.
        for kc in range(KC):
            for m in range(8):
                nc.tensor.matmul(
                    pg[:, m, s, :],
                    whh[l][:, kc, m * 128:(m + 1) * 128],
                    hbf[l][:, sp, kc, :],
                    start=False, stop=(s == TC - 1 and kc == KC - 1),
                    skip_group_check=True)
        if ABLATE != "nogates":
            nc.scalar.activation(rz[:], pg[:, 0:8, s, :], Sigmoid)
        for kc in range(KC):
            for m in range(8, MCH):
                nc.tensor.matmul(
                    pg[:, m + 4, s, :],
                    whh[l][:, kc, m * 128:(m + 1) * 128],
                    hbf[l][:, sp, kc, :],
                    start=False, stop=(s == TC - 1 and kc == KC - 1),
                    skip_group_check=True)
        if ABLATE == "nogates":
            return
        # h'_new = (1-w)h'_prev + w*s',  s' = sigma(2*aN)
        whp = tmp.tile([128, KC, BS], dt.float32, tag=f"whp{l}")
        nc.vector.tensor_mul(whp[:], rz[:, 4:8, :], hbf[l][:, sp, :, :])
        u = tmp.tile([128, KC, BS], dt.float32, tag=f"u{l}")
        nc.vector.tensor_sub(u[:], hbf[l][:, sp, :, :], whp[:])
        rn = tmp.tile([128, KC, BS], dt.float32, tag=f"rn{l}")
        aN = tmp.tile([128, KC, BS], dt.float32, tag=f"aN{l}")
        n = tmp.tile([128, KC, BS], dt.float32, tag=f"n{l}")
        v = tmp.tile([128, KC, BS], dt.float32, tag=f"v{l}")
        for ha, hb in ((0, 2), (2, 4)):
            nc.vector.tensor_mul(rn[:, ha:hb, :], pg[:, 12 + ha:12 + hb, s, :],
                                 rz[:, ha:hb, :])
            nc.vector.tensor_add(aN[:, ha:hb, :], rn[:, ha:hb, :],
                                 pg[:, 8 + ha:8 + hb, s, :])
            nc.scalar.activation(n[:, ha:hb, :], aN[:, ha:hb, :], Sigmoid,
                                 scale=2.0)
            nc.vector.tensor_mul(v[:, ha:hb, :], rz[:, 4 + ha:4 + hb, :],
                                 n[:, ha:hb, :])
            nc.vector.tensor_add(hbf[l][:, s, ha:hb, :], u[:, ha:hb, :],
                                 v[:, ha:hb, :])

    def out_epilogue(ci):
        mch = stream.tile([128, TC, 1, BS], dt.float16, tag="maskch")
        nc.sync.dma_start(mch[:, :, 0, :], mask[:, bass.ds(ci, TC), :])
        # out = (2h'-1)*m = (h'-0.5)*mask2 with mask2 in {0,2}
        uo = outp.tile([128, TC, KC, BS], dt.float16, tag="uout")
        nc.vector.tensor_scalar_sub(uo[:], hbf[2][:], 0.5)
        osb = outp.tile([128, TC, KC, BS], dt.float32, tag="osb")
        nc.vector.tensor_mul(osb[:], uo[:],
                             mch[:].broadcast_to([128, TC, KC, BS]))
        nc.sync.dma_start(io["outT"][:, bass.ds(ci, TC), :, :], osb[:])

    def slot(c0, c1, c2):
        pgs = {}
        if c0 is not None:
            pgs[0] = layer_setup(0, c0)
        if c1 is not None:
            pgs[1] = layer_setup(1, c1)
        if c2 is not None:
            pgs[2] = layer_setup(2, c2)
        for s in range(TC):
            for l in (0, 1, 2):
                if l in pgs:
                    layer_tick(l, s, pgs[l])
        if c2 is not None:
            out_epilogue(c2)

    full_cm = tc.For_i(0, RFULL, 1, name="rfull") if RFULL else None
    if full_cm is not None:
        full_cm.__enter__()

    for l in range(3):
        # h'-space: h=0 -> h'=0.5
        nc.gpsimd.memset(hbf[l][:, TC - 1, :, :], 0.5)

    slot(0, None, None)
    slot(TC, 0, None)
    hint = (mybir.EngineType.PE,)
    with tc.For_i(2 * TC, T, TC, hint_engines=hint, name="slots") as i:
        slot(i, i - TC, i - 2 * TC)
    slot(None, T - TC, T - 2 * TC)
    slot(None, None, T - TC)

    if full_cm is not None:
        full_cm.__exit__(None, None, None)
    ctx.close()


_CACHE = {}


def _get_program():
    if "nc" not in _CACHE:
        _CACHE["nc"] = _build_program()
    return _CACHE["nc"]


def _prep_shared(fc_w, fc_b, ws):
    sh = {}
    sh["fcwT"] = np.ascontiguousarray(
        fc_w.T.reshape(2, 128, HID)).astype(BF)
    sh["fcb"] = np.ascontiguousarray(fc_b.reshape(KC0, 128, 1)).astype(F32)
    # h'-space (h' = (h+1)/2): hidden-state inputs are h', so W -> 2W and
    # bias -= rowsum(W). Layer-0's input x0 is NOT h'-space (wih0 unscaled).
    # The z-gate rows (512:1024) are then negated so the PSUM holds -az and
    # a single sigmoid over rows 0:8 yields [r, w=1-z] in one ACT op.
    for l in range(3):
        w_ih, w_hh, _, _ = ws[l]
        kcl = KC0 if l == 0 else KC
        wih_eff = (w_ih if l == 0 else 2.0 * w_ih).copy()
        whh_eff = 2.0 * w_hh
        wih_eff[H:2 * H] *= -1.0
        whh_eff[H:2 * H] *= -1.0
        sh[f"wihT{l}"] = np.ascontiguousarray(
            wih_eff.T.reshape(kcl, 128, G3)).astype(BF)
        sh[f"whhT{l}"] = np.ascontiguousarray(
            whh_eff.T.reshape(KC, 128, G3)).astype(BF)
    bt = np.zeros((3, 16, 128), F32)
    for l in range(3):
        w_ih, w_hh, b_ih, b_hh = ws[l]
        bi_eff = b_ih - (0.0 if l == 0 else w_ih.sum(1))
        bh_eff = b_hh - w_hh.sum(1)
        bi = bi_eff.reshape(MCH, 128)
        bh = bh_eff.reshape(MCH, 128)
        bt[l, 0:8] = bi[0:8] + bh[0:8]
        bt[l, 4:8] *= -1.0
        bt[l, 8:12] = bi[8:12]
        bt[l, 12:16] = bh[8:12]
    sh["biasT"] = np.broadcast_to(
        bt.reshape(1, -1), (4, 3 * 16 * 128)).astype(BF).copy()
    oh = np.zeros((4, TC, BS), F32)
    for k in range(BS):
        oh[k, :, k] = 1.0
    sh["onehotT"] = oh.reshape(4, TC * BS).astype(BF)
    sh["iota"] = np.broadcast_to(
        np.arange(T, dtype=F32)[None, :], (128, T)).copy()
    return sh


def kernel(z, seq_lens, chord_embedding, fc_w, fc_b,
           w_ih0, w_hh0, b_ih0, b_hh0,
           w_ih1, w_hh1, b_ih1, b_hh1,
           w_ih2, w_hh2, b_ih2, b_hh2):
    z = np.asarray(z, F32)
    chord = np.asarray(chord_embedding, F32)
    seq = np.asarray(seq_lens)
    ws = [(np.asarray(w_ih0, F32), np.asarray(w_hh0, F32),
           np.asarray(b_ih0, F32), np.asarray(b_hh0, F32)),
          (np.asarray(w_ih1, F32), np.asarray(w_hh1, F32),
           np.asarray(b_ih1, F32), np.asarray(b_hh1, F32)),
          (np.asarray(w_ih2, F32), np.asarray(w_hh2, F32),
           np.asarray(b_ih2, F32), np.asarray(b_hh2, F32))]

    in_maps = _make_in_maps(z, seq, chord, np.asarray(fc_w, F32),
                            np.asarray(fc_b, F32), ws)
    res = _execute(in_maps)
    return _assemble(res.results)


def _make_in_maps(z, seq, chord, fc_w, fc_b, ws):
    sh = _prep_shared(fc_w, fc_b, ws)
    in_maps = []
    for c in range(NCORES):
        bs = slice(c * BS, (c + 1) * BS)
        m = dict(sh)
        m["chordT"] = np.ascontiguousarray(
            (chord[bs].transpose(2, 1, 0) / 100.0)
            .reshape(KC0, 128, T, BS)).astype(BF)
        m["zT"] = np.ascontiguousarray(
            z[bs].T.reshape(2, 128, BS)).astype(BF)
        m["seqrep"] = np.broadcast_to(
            seq[bs].astype(F32)[None, :], (128, BS)).copy()
        in_maps.append(m)
    return in_maps


def _execute(in_maps, **kw):
    nc = _get_program()
    return bass_utils.run_bass_kernel_spmd(nc, in_maps, list(range(NCORES)), **kw)


def _assemble(results):
    out = np.empty((B, T, H), F32)
    for c in range(NCORES):
        outT = np.asarray(results[c]["outT"])       # [128,T,KC,BS]
        out[c * BS:(c + 1) * BS] = (
            outT.transpose(3, 1, 2, 0).reshape(BS, T, H))
    return out


# revision 23
# speedup vs baseline: 1.2320x; 1.0808x over previous
"""Trainium2 Bass kernel for nn_Decoder (fc + 3-layer GRU + mask).

Strategy: data-parallel over batch B=32 across 8 cores (4 samples/core),
with the three GRU layers WAVEFRONTED on each core: slot j processes
layer0 chunk j, layer1 chunk j-1, layer2 chunk j-2 (TC=16 steps/chunk),
with per-step emission interleaving so each layer's serial gate chain
(ACT/DVE latency ~2us/step) hides under the other layers' PE matmuls.
Inter-layer chunks are handed off directly through the per-layer hbf
SBUF step buffers (WAR deps make the consumer's gx reads precede the
producer's overwrites) — no DRAM round-trip.

All activations are Sigmoid (single act table, no per-step reload):
h'-space recurrence with h' = (h+1)/2, s' = sigma(2*aN) = (tanh+1)/2,
w = sigma(-az) = 1-z (z weight rows pre-negated host-side):
  h'_new = (1-w) h'_prev + w s'
Weights fold the change of variables: W -> 2W, bias -= rowsum(W).

PSUM pg[l] [128, 16, TC, BS] f32 per layer (4KB/partition x3):
  j 0..7  : gx_rz + biases + gh_rz   (step MMs accumulate; z negated)
  j 8..11 : gx_n + b_ih_n            (read-only per step)
  j 12..15: b_hh_n + gh_n            (step MMs accumulate)
"""

import os
import sys
from contextlib import ExitStack

for _p in ("/opt/trn_rl_repo",):
    if _p not in sys.path:
        sys.path.insert(0, _p)

import numpy as np
import ml_dtypes

import concourse.bass as bass
import concourse.bacc as bacc
import concourse.mybir as mybir
import concourse.tile as tile
from concourse import bass_utils

BF = np.float16
F32 = np.float32
dt = mybir.dt

NCORES = 8
B, T = 32, 512
BS = B // NCORES          # 4 samples per core
TC = 16                   # time-steps per chunk
NCHUNK = T // TC          # 32
HID = 1024                # layer-0 input dim
H = 512                   # GRU hidden
G3 = 3 * H                # 1536
MCH = G3 // 128           # 12 gate-dim chunks: r0..3 z0..3 n0..3
KC = H // 128             # 4 hidden chunks
KC0 = HID // 128          # 8 input chunks for layer 0
ABLATE = ""               # "nogates"
RFULL = 0                 # device-side repeats of the full body

Sigmoid = mybir.ActivationFunctionType.Sigmoid
Relu = mybir.ActivationFunctionType.Relu
Alu = mybir.AluOpType


def _declare_io(nc):
    d = {}
    inp = lambda n, s, t: nc.dram_tensor(n, s, t, kind="ExternalInput").ap()
    d["chordT"] = inp("chordT", [KC0, 128, T, BS], dt.float16)
    d["zT"] = inp("zT", [2, 128, BS], dt.float16)
    d["fcwT"] = inp("fcwT", [2, 128, HID], dt.float16)
    d["fcb"] = inp("fcb", [KC0, 128, 1], dt.float32)
    d["wihT0"] = inp("wihT0", [KC0, 128, G3], dt.float16)
    d["wihT1"] = inp("wihT1", [KC, 128, G3], dt.float16)
    d["wihT2"] = inp("wihT2", [KC, 128, G3], dt.float16)
    d["whhT0"] = inp("whhT0", [KC, 128, G3], dt.float16)
    d["whhT1"] = inp("whhT1", [KC, 128, G3], dt.float16)
    d["whhT2"] = inp("whhT2", [KC, 128, G3], dt.float16)
    d["biasT"] = inp("biasT", [4, 3 * 16 * 128], dt.float16)
    d["onehotT"] = inp("onehotT", [4, TC * BS], dt.float16)
    d["iota"] = inp("iota", [128, T], dt.float32)
    d["seqrep"] = inp("seqrep", [128, BS], dt.float32)
    d["outT"] = nc.dram_tensor("outT", [128, T, KC, BS], dt.float32,
                               kind="ExternalOutput").ap()
    return d


def _build_program(debug=False):
    nc = bacc.Bacc("TRN2", target_bir_lowering=False, debug=debug,
                   num_devices=NCORES)
    io = _declare_io(nc)
    with tile.TileContext(nc) as tc:
        _emit(tc, io)
    nc.compile()
    return nc


def _emit(tc, io):
    nc = tc.nc
    ctx = ExitStack()
    const = ctx.enter_context(tc.tile_pool(name="const", bufs=1))
    stream = ctx.enter_context(tc.tile_pool(name="stream", bufs=3))
    tmp = ctx.enter_context(tc.tile_pool(name="tmp", bufs=3))
    outp = ctx.enter_context(tc.tile_pool(name="outp", bufs=2))
    pgp = ctx.enter_context(tc.tile_pool(name="pg", bufs=1, space="PSUM"))

    # ---- persistent SBUF tensors -------------------------------------
    wih = [const.tile([128, KC0, G3], dt.float16, tag="wih0", name="wih0"),
           const.tile([128, KC, G3], dt.float16, tag="wih1", name="wih1"),
           const.tile([128, KC, G3], dt.float16, tag="wih2", name="wih2")]
    whh = [const.tile([128, KC, G3], dt.float16, tag=f"whh{l}",
                      name=f"whh{l}") for l in range(3)]
    mask = const.tile([128, T, BS], dt.float16, tag="mask")
    biasb = const.tile([4, 3, 16, 128], dt.float16, tag="biasb")
    onehot = const.tile([4, TC * BS], dt.float16, tag="onehot")
    fc_hT = const.tile([128, KC0, BS], dt.float16, tag="fchT")
    gfcT = const.tile([4, MCH, 128], dt.float16, tag="gfcT")
    biasc0 = const.tile([4, 16, 128], dt.float16, tag="biasc0")
    hbf = [const.tile([128, TC, KC, BS], dt.float16, tag=f"hbf{l}",
                      name=f"hbf{l}") for l in range(3)]

    # ---- load weights / constants ------------------------------------
    for kc in range(KC0):
        nc.sync.dma_start(wih[0][:, kc, :], io["wihT0"][kc])
    for l in (1, 2):
        for kc in range(KC):
            nc.sync.dma_start(wih[l][:, kc, :], io[f"wihT{l}"][kc])
    for l in range(3):
        for kc in range(KC):
            nc.sync.dma_start(whh[l][:, kc, :], io[f"whhT{l}"][kc])
    nc.sync.dma_start(
        biasb[:].rearrange("p l j g -> p (l j g)"), io["biasT"])
    nc.sync.dma_start(onehot[:], io["onehotT"])

    # ---- prologue: mask, fc, gfcT ------------------------------------
    with ExitStack() as pctx:
        psb = pctx.enter_context(tc.tile_pool(name="psb", bufs=2))
        pps = pctx.enter_context(tc.tile_pool(name="pps", bufs=1,
                                              space="PSUM"))

        iota_sb = psb.tile([128, T], dt.float32, tag="iota")
        seq_sb = psb.tile([128, BS], dt.float32, tag="seq")
        nc.sync.dma_start(iota_sb[:], io["iota"])
        nc.sync.dma_start(seq_sb[:], io["seqrep"])
        for b in range(BS):
            # {0,2}-valued: out = 2*(t < seq); folds the h = 2h'-1 output
            # rescale into the mask (h'-space kernel)
            nc.vector.tensor_scalar(mask[:, :, b], iota_sb[:],
                                    seq_sb[:, b:b + 1], 2.0, op0=Alu.is_lt,
                                    op1=Alu.mult)

        z_sb = psb.tile([128, 2, BS], dt.float16, tag="zsb")
        fcw_sb = psb.tile([128, 2, HID], dt.float16, tag="fcw")
        fcb_sb = psb.tile([128, KC0], dt.float32, tag="fcb")
        for kc in range(2):
            nc.sync.dma_start(z_sb[:, kc, :], io["zT"][kc])
            nc.sync.dma_start(fcw_sb[:, kc, :], io["fcwT"][kc])
        for hc in range(KC0):
            nc.sync.dma_start(fcb_sb[:, hc:hc + 1], io["fcb"][hc])
        for hc in range(KC0):
            pfc = pps.tile([128, BS], dt.float32, tag="pfc")
            for kc in range(2):
                nc.tensor.matmul(pfc[:], fcw_sb[:, kc, hc * 128:(hc + 1) * 128],
                                 z_sb[:, kc, :], start=(kc == 0), stop=(kc == 1))
            nc.scalar.activation(fc_hT[:, hc, :], pfc[:], Relu,
                                 bias=fcb_sb[:, hc:hc + 1], scale=1.0)
        # gfcT[b, m*128+g] = (fc_h^T @ w_ih0^T): layer-0 time-constant term
        for m in range(MCH):
            pgf = pps.tile([4, 128], dt.float32, tag="pgf")
            for kc in range(KC0):
                nc.tensor.matmul(pgf[:], fc_hT[:, kc, :],
                                 wih[0][:, kc, m * 128:(m + 1) * 128],
                                 start=(kc == 0), stop=(kc == KC0 - 1))
            nc.vector.tensor_copy(gfcT[:, m, :], pgf[:])
        # layer-0 combined per-sample seed: bias + fc term (j 0..11), so
        # the per-slot gfc MMs disappear
        nc.vector.tensor_add(biasc0[:, 0:MCH, :], biasb[:, 0, 0:MCH, :],
                             gfcT[:])
        nc.vector.tensor_copy(biasc0[:, MCH:16, :], biasb[:, 0, MCH:16, :])

    # ---- wavefront slot loop -----------------------------------------
    def layer_setup(l, ci):
        """Chunk setup for layer l at chunk offset ci (element units):
        gx GEMM + bias (+fc for l=0) seeded into PSUM."""
        pg = pgp.tile([128, 16, TC, BS], dt.float32, tag=f"pg{l}")
        if l == 0:
            chd = stream.tile([128, TC, KC0, BS], dt.float16, tag="chd")
            for kc in range(KC0):
                nc.sync.dma_start(chd[:, :, kc, :],
                                  io["chordT"][kc, :, bass.ds(ci, TC), :])
            srcap = lambda kc: chd[:, :, kc, :]
            kcl = KC0
        else:
            srcap = lambda kc: hbf[l - 1][:, :, kc, :]
            kcl = KC
        # pg is 4KB/partition = 2 PSUM banks (j0..7 / j8..15); start=True
        # clears the WHOLE bank, so only the first writer of each bank
        # (gx m=0 and m=8, kc=0) may set it.
        for m in range(MCH):
            for kc in range(kcl):
                st = (kc == 0) and (m in (0, 8))
                nc.tensor.matmul(
                    pg[:, m, :, :],
                    wih[l][:, kc, m * 128:(m + 1) * 128],
                    srcap(kc), start=st, stop=False,
                    skip_group_check=True)
        for j in range(16):
            bsrc = biasc0[:, j, :] if l == 0 else biasb[:, l, j, :]
            nc.tensor.matmul(
                pg[:, j, :, :], bsrc, onehot[:],
                start=False, stop=False, skip_group_check=True)
        return pg

    def layer_tick(l, s, pg):
        sp = (s - 1) % TC
        if ABLATE == "nochain":
            sp = TC - 1
        rz = tmp.tile([128, 8, BS], dt.float32, tag=f"rz{l}")
        # PE order: rz-block, n-block, both kc-outer so the next tick's
        # kc01 MMs only need the kc01 half of h. sigma_rw fires after the
        # rz block (one ACT op: z rows pre-negated so rz[4:8]=w=1-z) and
        # hides under the n block; whp/u hide under sigma2. The post-MM
        # chain is split in kc halves so h[kc01] lands one half-chain
        # early and the next tick's PE starts sooner.
        for kc in range(KC):
            for m in range(8):
                nc.tensor.matmul(
                    pg[:, m, s, :],
                    whh[l][:, kc, m * 128:(m + 1) * 128],
                    hbf[l][:, sp, kc, :],
                    start=False, stop=(s == TC - 1 and kc == KC - 1),
                    skip_group_check=True)
        if ABLATE != "nogates":
            nc.scalar.activation(rz[:], pg[:, 0:8, s, :], Sigmoid)
        for kc in range(KC):
            for m in range(8, MCH):
                nc.tensor.matmul(
                    pg[:, m + 4, s, :],
                    whh[l][:, kc, m * 128:(m + 1) * 128],
                    hbf[l][:, sp, kc, :],
                    start=False, stop=(s == TC - 1 and kc == KC - 1),
                    skip_group_check=True)
        if ABLATE == "nogates":
            return
        # h'_new = (1-w)h'_prev + w*s',  s' = sigma(2*aN)
        whp = tmp.tile([128, KC, BS], dt.float32, tag=f"whp{l}")
        nc.vector.tensor_mul(whp[:], rz[:, 4:8, :], hbf[l][:, sp, :, :])
        u = tmp.tile([128, KC, BS], dt.float32, tag=f"u{l}")
        nc.vector.tensor_sub(u[:], hbf[l][:, sp, :, :], whp[:])
        rn = tmp.tile([128, KC, BS], dt.float32, tag=f"rn{l}")
        aN = tmp.tile([128, KC, BS], dt.float32, tag=f"aN{l}")
        n = tmp.tile([128, KC, BS], dt.float32, tag=f"n{l}")
        v = tmp.tile([128, KC, BS], dt.float32, tag=f"v{l}")
        for ha, hb in ((0, 2), (2, 4)):
            nc.vector.tensor_mul(rn[:, ha:hb, :], pg[:, 12 + ha:12 + hb, s, :],
                                 rz[:, ha:hb, :])
            nc.vector.tensor_add(aN[:, ha:hb, :], rn[:, ha:hb, :],
                                 pg[:, 8 + ha:8 + hb, s, :])
            nc.scalar.activation(n[:, ha:hb, :], aN[:, ha:hb, :], Sigmoid,
                                 scale=2.0)
            nc.vector.tensor_mul(v[:, ha:hb, :], rz[:, 4 + ha:4 + hb, :],
                                 n[:, ha:hb, :])
            nc.vector.tensor_add(hbf[l][:, s, ha:hb, :], u[:, ha:hb, :],
                                 v[:, ha:hb, :])

    def out_epilogue(ci):
        mch = stream.tile([128, TC, 1, BS], dt.float16, tag="maskch")
        nc.sync.dma_start(mch[:, :, 0, :], mask[:, bass.ds(ci, TC), :])
        # out = (2h'-1)*m = (h'-0.5)*mask2 with mask2 in {0,2}
        uo = outp.tile([128, TC, KC, BS], dt.float16, tag="uout")
        nc.vector.tensor_scalar_sub(uo[:], hbf[2][:], 0.5)
        osb = outp.tile([128, TC, KC, BS], dt.float32, tag="osb")
        nc.vector.tensor_mul(osb[:], uo[:],
                             mch[:].broadcast_to([128, TC, KC, BS]))
        nc.sync.dma_start(io["outT"][:, bass.ds(ci, TC), :, :], osb[:])

    def slot(c0, c1, c2):
        pgs = {}
        if c0 is not None:
            pgs[0] = layer_setup(0, c0)
        if c1 is not None:
            pgs[1] = layer_setup(1, c1)
        if c2 is not None:
            pgs[2] = layer_setup(2, c2)
        for s in range(TC):
            for l in (0, 1, 2):
                if l in pgs:
                    layer_tick(l, s, pgs[l])
        if c2 is not None:
            out_epilogue(c2)

    full_cm = tc.For_i(0, RFULL, 1, name="rfull") if RFULL else None
    if full_cm is not None:
        full_cm.__enter__()

    for l in range(3):
        # h'-space: h=0 -> h'=0.5
        nc.gpsimd.memset(hbf[l][:, TC - 1, :, :], 0.5)

    slot(0, None, None)
    slot(TC, 0, None)
    hint = (mybir.EngineType.PE,)
    with tc.For_i(2 * TC, T, 2 * TC, hint_engines=hint, name="slots") as i:
        slot(i, i - TC, i - 2 * TC)
        slot(i + TC, i, i - TC)
    slot(None, T - TC, T - 2 * TC)
    slot(None, None, T - TC)

    if full_cm is not None:
        full_cm.__exit__(None, None, None)
    ctx.close()


_CACHE = {}


def _get_program():
    if "nc" not in _CACHE:
        _CACHE["nc"] = _build_program()
    return _CACHE["nc"]


def _prep_shared(fc_w, fc_b, ws):
    sh = {}
    sh["fcwT"] = np.ascontiguousarray(
        fc_w.T.reshape(2, 128, HID)).astype(BF)
    sh["fcb"] = np.ascontiguousarray(fc_b.reshape(KC0, 128, 1)).astype(F32)
    # h'-space (h' = (h+1)/2): hidden-state inputs are h', so W -> 2W and
    # bias -= rowsum(W). Layer-0's input x0 is NOT h'-space (wih0 unscaled).
    # The z-gate rows (512:1024) are then negated so the PSUM holds -az and
    # a single sigmoid over rows 0:8 yields [r, w=1-z] in one ACT op.
    for l in range(3):
        w_ih, w_hh, _, _ = ws[l]
        kcl = KC0 if l == 0 else KC
        wih_eff = (w_ih if l == 0 else 2.0 * w_ih).copy()
        whh_eff = 2.0 * w_hh
        wih_eff[H:2 * H] *= -1.0
        whh_eff[H:2 * H] *= -1.0
        sh[f"wihT{l}"] = np.ascontiguousarray(
            wih_eff.T.reshape(kcl, 128, G3)).astype(BF)
        sh[f"whhT{l}"] = np.ascontiguousarray(
            whh_eff.T.reshape(KC, 128, G3)).astype(BF)
    bt = np.zeros((3, 16, 128), F32)
    for l in range(3):
        w_ih, w_hh, b_ih, b_hh = ws[l]
        bi_eff = b_ih - (0.0 if l == 0 else w_ih.sum(1))
        bh_eff = b_hh - w_hh.sum(1)
        bi = bi_eff.reshape(MCH, 128)
        bh = bh_eff.reshape(MCH, 128)
        bt[l, 0:8] = bi[0:8] + bh[0:8]
        bt[l, 4:8] *= -1.0
        bt[l, 8:12] = bi[8:12]
        bt[l, 12:16] = bh[8:12]
    sh["biasT"] = np.broadcast_to(
        bt.reshape(1, -1), (4, 3 * 16 * 128)).astype(BF).copy()
    oh = np.zeros((4, TC, BS), F32)
    for k in range(BS):
        oh[k, :, k] = 1.0
    sh["onehotT"] = oh.reshape(4, TC * BS).astype(BF)
    sh["iota"] = np.broadcast_to(
        np.arange(T, dtype=F32)[None, :], (128, T)).copy()
    return sh


def kernel(z, seq_lens, chord_embedding, fc_w, fc_b,
           w_ih0, w_hh0, b_ih0, b_hh0,
           w_ih1, w_hh1, b_ih1, b_hh1,
           w_ih2, w_hh2, b_ih2, b_hh2):
    z = np.asarray(z, F32)
    chord = np.asarray(chord_embedding, F32)
    seq = np.asarray(seq_lens)
    ws = [(np.asarray(w_ih0, F32), np.asarray(w_hh0, F32),
           np.asarray(b_ih0, F32), np.asarray(b_hh0, F32)),
          (np.asarray(w_ih1, F32), np.asarray(w_hh1, F32),
           np.asarray(b_ih1, F32), np.asarray(b_hh1, F32)),
          (np.asarray(w_ih2, F32), np.asarray(w_hh2, F32),
           np.asarray(b_ih2, F32), np.asarray(b_hh2, F32))]

    in_maps = _make_in_maps(z, seq, chord, np.asarray(fc_w, F32),
                            np.asarray(fc_b, F32), ws)
    res = _execute(in_maps)
    return _assemble(res.results)


def _make_in_maps(z, seq, chord, fc_w, fc_b, ws):
    sh = _prep_shared(fc_w, fc_b, ws)
    in_maps = []
    for c in range(NCORES):
        bs = slice(c * BS, (c + 1) * BS)
        m = dict(sh)
        m["chordT"] = np.ascontiguousarray(
            (chord[bs].transpose(2, 1, 0) / 100.0)
            .reshape(KC0, 128, T, BS)).astype(BF)
        m["zT"] = np.ascontiguousarray(
            z[bs].T.reshape(2, 128, BS)).astype(BF)
        m["seqrep"] = np.broadcast_to(
            seq[bs].astype(F32)[None, :], (128, BS)).copy()
        in_maps.append(m)
    return in_maps


def _execute(in_maps, **kw):
    nc = _get_program()
    return bass_utils.run_bass_kernel_spmd(nc, in_maps, list(range(NCORES)), **kw)


def _assemble(results):
    out = np.empty((B, T, H), F32)
    for c in range(NCORES):
        outT = np.asarray(results[c]["outT"])       # [128,T,KC,BS]
        out[c * BS:(c + 1) * BS] = (
            outT.transpose(3, 1, 2, 0).reshape(BS, T, H))
    return out
